# revision 1
# baseline (speedup 1.0000x reference)
"""Trainium2 Bass kernel for causal self-attention with PoPE.

Reference computation (B=2, T=2048, C=1024, H=16, D=64):
  qkv = x @ w_attn.T ; split q,k,v ; heads
  mu_q = softplus(q); mu_k = softplus(k)
  q_real = mu_q * cos(t w); q_imag = mu_q * sin(t w)
  k_real = mu_k * cos(t w + d); k_imag = mu_k * sin(t w + d)   [d = clip(delta)]
  att = softmax_causal((q_real k_real + q_imag k_imag)/sqrt(D))
  y = att @ v ; out = y @ w_proj.T

Sharding: 8 cores = 2 batches x 4 head-groups (4 heads each). Each core
computes its batch's QKV for its heads, attention, and a partial c_proj
(its heads' input-channel rows of w_proj). Host sums the 4 partials per
batch.

Per-core dataflow (all matmuls float32r: full PE rate at free dim >=
256, ~FP22 mantissa):
  xT   [c, t]     x[b]^T, c on partitions (8 tiles of 128)
  qk_h [128, t]   rows 0:64 = q_h, 64:128 = k_h (d-major), psum
  mu_h = ln(exp(qk_h)+1)   (ACT, exp in-place on psum; same table set
                            as the attention exp -> no table switches)
  Qt_h [128, t]   rows 0:64 mu_q*cos(tw)/8, rows 64:128 mu_q*sin(tw)/8
  Kt_h [128, t]   rows 0:64 mu_k*cos(tw+d), rows 64:128 mu_k*sin(tw+d)
  S^T  [tk, tq]   = Kt^T @ Qt (single K=128 matmul per 128x512 block,
                   two tq blocks share one 2-bank psum tile)
  P = exp(S^T)    no max subtraction (scores bounded ~6); causal mask as
                  0/1 multiply on the 16 diagonal blocks only (gpsimd)
  y_aug^T [96,tq] += V_aug[tk]^T @ P : V_aug = [V | ones | zeros] so row
                  64 of the psum accumulates the softmax denominator
  normalize       recip(denom) -> PE outer-product broadcast -> multiply
  c_proj          out[t,e] psum += y_t[c,t]^T @ w_projT[c,e]

Attention loops j-pairs outer so only 2 y-psum banks are live, leaving
room to double-buffer the 2-bank S tiles (fewer, wider ACT exp ops).
"""

import math
import os
import sys

import numpy as np

for _p in ("/opt/trn_rl_repo",):
    if _p not in sys.path and os.path.isdir(_p):
        sys.path.insert(0, _p)

import concourse.tile as tile
from concourse import bacc
from concourse import mybir
from concourse import bass_utils

B, T, C = 2, 2048, 1024
H, D = 16, 64
BASE = 10000.0
N_CORES = 8
HPC = 4  # heads per core
NCT = 8  # c tiles (1024/128)
NTT = 16  # t tiles of 128

F32 = mybir.dt.float32
F32R = mybir.dt.float32r
AF = mybir.ActivationFunctionType


def build_module():
    nc = bacc.Bacc(
        "TRN2", target_bir_lowering=False, debug=False, num_devices=N_CORES
    )

    xT_d = nc.dram_tensor("xT", (NCT, 128, T), F32R, kind="ExternalInput").ap()
    wqk_d = nc.dram_tensor("wqk", (NCT, 128, 512), F32R, kind="ExternalInput").ap()
    wv_d = nc.dram_tensor("wv", (NCT, 128, 256), F32R, kind="ExternalInput").ap()
    w2t_d = nc.dram_tensor("w2t", (2, 128, 1024), F32R, kind="ExternalInput").ap()
    trig_d = nc.dram_tensor("trig", (128, T), F32, kind="ExternalInput").ap()
    ab_d = nc.dram_tensor("ab", (HPC, 128, T), F32, kind="ExternalInput").ap()
    cmask_d = nc.dram_tensor("cmask", (128, 128), F32, kind="ExternalInput").ap()
    out_d = nc.dram_tensor("out", (NTT, 128, 1024), F32, kind="ExternalOutput").ap()

    idm = list(range(32))

    with tile.TileContext(nc) as tc:
        with (
            tc.tile_pool(name="persist", bufs=1) as persist,
            tc.tile_pool(name="mupool", bufs=1) as mupool,
            tc.tile_pool(name="ps2", bufs=2, space="PSUM") as ps2,
            tc.tile_pool(name="ps1", bufs=4, space="PSUM") as ps1,
        ):
            # persistent tiles (live across phases)
            v_aug = persist.tile([128, NTT, HPC, 65], F32R)
            ones_t = persist.tile([128, 128], F32R)
            nc.vector.memset(ones_t.bitcast(F32), 1.0)
            # per head slab: cols 0:64 = V, col 64 = ones (the y matmul
            # then accumulates the softmax denominator in psum row 64)
            nc.vector.memset(
                v_aug.rearrange("p a b c -> p (a b) c")[:, :, 64:65].bitcast(F32),
                1.0,
            )

            trig = persist.tile([128, T], F32)
            nc.gpsimd.dma_start(trig, trig_d)

            mu = [mupool.tile([128, T], F32, name=f"mu{h}") for h in range(HPC)]

            # ---------------- Phase A: QKV projection ----------------
            with tc.tile_pool(name="phA", bufs=1) as pha:
                xT = pha.tile([128, NCT, T], F32R)
                wv = pha.tile([128, NCT, 256], F32R)
                wqk_pool = tc.tile_pool(name="wqkp", bufs=1)
                wqkp = wqk_pool.__enter__()
                wqk = wqkp.tile([128, NCT, 512], F32R)
                nc.scalar.dma_start(wqk, wqk_d.rearrange("o p e -> p o e"))
                engs = [nc.sync, nc.gpsimd, nc.scalar]
                for o in range(NCT):
                    engs[o % 3].dma_start(xT[:, o, :], xT_d[o])
                nc.sync.dma_start(wv, wv_d.rearrange("o p e -> p o e"))

                # q,k per head: 2-bank psum [128, 1024] per tb-pair.
                # softplus = ln(exp(x)+1): exp lands in mu, ln runs in
                # place afterwards, batched so the ACT table isn't
                # reloaded between alternating Exp/Ln ops.
                exp_handles = {}
                for h in range(HPC):
                    for tbp in range(2):
                        ps = ps2.tile([128, 1024], F32, tag="s2", name="ps_qk")
                        base = tbp * 1024
                        for c in range(NCT):
                            for half in range(2):
                                o0 = half * 512
                                nc.tensor.matmul(
                                    ps[:, o0 : o0 + 512],
                                    lhsT=wqk[:, c, h * 128 : (h + 1) * 128],
                                    rhs=xT[:, c, base + o0 : base + o0 + 512],
                                    start=(c == 0),
                                    stop=(c == NCT - 1),
                                )
                        exp_handles[(h, tbp)] = nc.scalar.activation(
                            mu[h][:, base : base + 1024], ps, AF.Exp
                        )
                from concourse.tile_rust import add_dep_helper

                for h in range(HPC):
                    for tbp in range(2):
                        base = tbp * 1024
                        ln = nc.scalar.activation(
                            mu[h][:, base : base + 1024],
                            mu[h][:, base : base + 1024],
                            AF.Ln,
                            bias=1.0,
                        )
                        # order Lns after the 2-head group's last Exp so the
                        # ACT table isn't reloaded between every Exp/Ln pair
                        grp_last = exp_handles[(h, 1)]
                        add_dep_helper(
                            ln.ins,
                            grp_last.ins,
                            sync=False,
                            reason="group softplus lns after exps (ACT tables)",
                        )

                wqk_pool.__exit__(None, None, None)

                # V in [t, e] layout -> v_aug columns 0:64 per head
                for tt in range(NTT):
                    psv = ps1.tile([128, 512], F32, tag="mm", name="ps_v")
                    for c in range(NCT):
                        nc.tensor.matmul(
                            psv[:, 0:256],
                            lhsT=xT[:, c, tt * 128 : (tt + 1) * 128],
                            rhs=wv[:, c, :],
                            start=(c == 0),
                            stop=(c == NCT - 1),
                        )
                    nc.vector.tensor_copy(
                        out=v_aug[:, tt, :, 0:64],
                        in_=psv[:, 0:256].rearrange("p (h e) -> p h e", h=HPC),
                    )

            # ------------- Phase B: attention, j-pairs outer -------------
            ytp_ctx = tc.tile_pool(name="ytp", bufs=1)
            ytp = ytp_ctx.__enter__()
            y_t = ytp.tile([128, 2, T], F32R)
            w2 = ytp.tile([128, 2, 1024], F32R)
            nc.sync.dma_start(w2, w2t_d.rearrange("o p e -> p o e"))
            with (
                tc.tile_pool(name="phB", bufs=1) as phb,
                tc.tile_pool(name="abp", bufs=2) as abp,
                tc.tile_pool(name="qp", bufs=3) as qp,
                tc.tile_pool(name="kp", bufs=3) as kp,
                tc.tile_pool(name="swp", bufs=2) as swp,
                tc.tile_pool(name="pp", bufs=7) as pp,
                tc.tile_pool(name="smalls", bufs=3) as smalls,
            ):
                cmask = phb.tile([128, 128], F32)
                nc.gpsimd.dma_start(cmask, cmask_d)
                for h in range(HPC):
                    abh = abp.tile([128, T], F32, tag="ab", name="abh")
                    nc.sync.dma_start(abh, ab_d[h])
                    qt = qp.tile([128, T], F32R, tag="qt", name="qt")
                    kt = kp.tile([128, T], F32R, tag="kt", name="kt")
                    # cross-partition moves through f32 scratch (shuffle
                    # can't write f32r; TensorTensor needs aligned bases)
                    musw = swp.tile([128, T], F32, tag="musw", name="musw")
                    for hb in range(2):
                        ts_ = slice(hb * 1024, hb * 1024 + 1024)
                        nc.vector.stream_shuffle(
                            musw[64:128, ts_], mu[h][0:64, ts_], idm
                        )
                        nc.vector.stream_shuffle(
                            musw[0:64, ts_], mu[h][64:128, ts_], idm
                        )
                        # shuffle-dependent halves on DVE (fast, critical
                        # path); independent halves on gpsimd (2x slower but
                        # off the critical path)
                        nc.vector.tensor_mul(
                            kt[0:64, ts_], musw[0:64, ts_], abh[0:64, ts_]
                        )
                        nc.gpsimd.tensor_mul(
                            kt[64:128, ts_], mu[h][64:128, ts_], abh[64:128, ts_]
                        )
                        nc.gpsimd.tensor_mul(
                            qt[0:64, ts_], mu[h][0:64, ts_], trig[0:64, ts_]
                        )
                        nc.vector.tensor_mul(
                            qt[64:128, ts_], musw[64:128, ts_], trig[64:128, ts_]
                        )

                    for jp in range(2):
                        j0, j1 = 2 * jp, 2 * jp + 1
                        yps = [
                            ps1.tile([128, 512], F32, tag="mm", name=f"ps_y{jj}")
                            for jj in range(2)
                        ]
                        imax = 4 * j1 + 3
                        for i in range(imax + 1):
                            jlo = i // 4  # lowest valid j for this tk tile
                            r = i % 4
                            jset = [j for j in (j0, j1) if j >= jlo]
                            # for the diagonal block (j == jlo) only columns
                            # >= 128*r can be causally valid: narrow the S
                            # matmul, exp and y matmul to that range; the
                            # skipped psum columns get no contribution from
                            # this tk tile, which is exactly correct.
                            sps = ps2.tile([128, 1024], F32, tag="s2", name="ps_s")
                            for j in jset:
                                o0 = (j - j0) * 512
                                lo = 128 * r if j == jlo else 0
                                nc.tensor.matmul(
                                    sps[:, o0 + lo : o0 + 512],
                                    lhsT=kt[:, i * 128 : (i + 1) * 128],
                                    rhs=qt[:, j * 512 + lo : (j + 1) * 512],
                                    start=True,
                                    stop=True,
                                )
                            p_sb = pp.tile([128, 1024], F32R, tag="p", name="p_sb")
                            c0 = (jset[0] - j0) * 512 + (
                                128 * r if jset[0] == jlo else 0
                            )
                            c1 = (jset[-1] - j0) * 512 + 512
                            nc.scalar.activation(
                                p_sb[:, c0:c1], sps[:, c0:c1], AF.Exp
                            )
                            if jlo in (j0, j1):
                                # mask only the 128-wide diagonal strip
                                boff = (jlo - j0) * 512 + 128 * r
                                nc.gpsimd.tensor_mul(
                                    p_sb[:, boff : boff + 128],
                                    p_sb[:, boff : boff + 128],
                                    cmask,
                                )
                            for j in jset:
                                o0 = (j - j0) * 512
                                lo = 128 * r if j == jlo else 0
                                nc.tensor.matmul(
                                    yps[j - j0][0:65, lo:512],
                                    lhsT=v_aug[:, i, h, :],
                                    rhs=p_sb[:, o0 + lo : o0 + 512],
                                    start=(i == 0),
                                    stop=(i == 4 * j + 3),
                                )
                        for jj, j in ((0, j0), (1, j1)):
                            # reciprocal directly on psum row 64 (same start
                            # partition for in and out keeps the ISA happy)
                            rc = smalls.tile([128, 512], F32R, tag="rc", name="rc")
                            with nc.allow_low_precision(
                                reason="f32r (~fp22) reciprocal of softmax denom"
                            ):
                                nc.vector.reciprocal(
                                    rc[64:65, :], yps[jj][64:65, :]
                                )
                            # broadcast across partitions via PE outer
                            # product: ones[1,128].T @ rc[1,512] -> psum
                            bps = ps1.tile([128, 512], F32, tag="mm", name="ps_bc")
                            nc.tensor.matmul(
                                bps,
                                lhsT=ones_t[64:65, :],
                                rhs=rc[64:65, :],
                                start=True,
                                stop=True,
                            )
                            bc = smalls.tile([128, 512], F32, tag="bc", name="bc")
                            nc.vector.tensor_copy(out=bc, in_=bps)
                            if h % 2 == 0:
                                nc.vector.tensor_mul(
                                    y_t[0:64, h // 2, j * 512 : (j + 1) * 512],
                                    yps[jj][0:64, :],
                                    bc[0:64, :],
                                )
                            else:
                                # odd heads land on partitions 64:128 of y_t
                                ysh = smalls.tile(
                                    [128, 512], F32, tag="ysh", name="ysh"
                                )
                                nc.vector.stream_shuffle(
                                    ysh[64:128, :], yps[jj][0:64, :], idm
                                )
                                nc.vector.tensor_mul(
                                    y_t[64:128, h // 2, j * 512 : (j + 1) * 512],
                                    ysh[64:128, :],
                                    bc[64:128, :],
                                )

            # ---------------- Phase C: output projection ----------------
            with tc.tile_pool(name="ostage", bufs=8) as ostage:
                for tt in range(NTT):
                    po = ps2.tile([128, 1024], F32, tag="s2", name="ps_o")
                    for ct in range(2):
                        for eh in range(2):
                            nc.tensor.matmul(
                                po[:, eh * 512 : eh * 512 + 512],
                                lhsT=y_t[:, ct, tt * 128 : (tt + 1) * 128],
                                rhs=w2[:, ct, eh * 512 : (eh + 1) * 512],
                                start=(ct == 0),
                                stop=(ct == 1),
                            )
                    ost = ostage.tile([128, 1024], F32, tag="o", name="ost")
                    if tt % 2 == 0:
                        nc.scalar.copy(ost, po)
                    else:
                        nc.vector.tensor_copy(out=ost, in_=po)
                    eng = nc.sync if tt % 2 == 0 else nc.gpsimd
                    eng.dma_start(out_d[tt], ost)

            ytp_ctx.__exit__(None, None, None)

    nc.compile()
    return nc


def make_inputs(x, w_attn, w_proj, delta):
    """Host-side prep: per-core input dicts (core = b*4 + g)."""
    x = np.asarray(x, dtype=np.float32)
    w_attn = np.asarray(w_attn, dtype=np.float32)
    w_proj = np.asarray(w_proj, dtype=np.float32)
    delta = np.asarray(delta, dtype=np.float32)

    inv_freq = 1.0 / (BASE ** (np.arange(D, dtype=np.float32) / D))
    t = np.arange(T, dtype=np.float32)
    freqs = t[:, None] * inv_freq[None, :]  # (T, D)
    scale = 1.0 / math.sqrt(D)
    cosTs = (np.cos(freqs).T * scale).astype(np.float32)  # (D, T)
    sinTs = (np.sin(freqs).T * scale).astype(np.float32)
    trig = np.concatenate([cosTs, sinTs], axis=0)  # (128, T)

    d = np.clip(delta, -2.0 * math.pi, 0.0)

    qw = w_attn[:C].reshape(H, D, C)
    kw = w_attn[C : 2 * C].reshape(H, D, C)
    vw = w_attn[2 * C :].reshape(H, D, C)

    # causal mask for the 128-wide diagonal strip: valid iff c >= tk
    tk = np.arange(128)[:, None]
    cc = np.arange(128)[None, :]
    cmask = (cc >= tk).astype(np.float32)

    in_maps = []
    for core in range(N_CORES):
        b, g = divmod(core, HPC)
        heads = range(HPC * g, HPC * g + HPC)

        xT = np.ascontiguousarray(x[b].T).reshape(NCT, 128, T)

        qk = np.stack(
            [np.concatenate([qw[h], kw[h]], axis=0) for h in heads], axis=0
        )  # (4, 128, C)
        wqk = np.ascontiguousarray(qk.transpose(2, 0, 1).reshape(C, 512)).reshape(
            NCT, 128, 512
        )
        wv = np.ascontiguousarray(
            vw[HPC * g : HPC * g + HPC].reshape(256, C).T
        ).reshape(NCT, 128, 256)
        w2t = np.ascontiguousarray(
            w_proj[:, 256 * g : 256 * (g + 1)].T
        ).reshape(2, 128, 1024)

        ab = np.stack(
            [
                np.concatenate(
                    [
                        np.cos(freqs + d[h][None, :]).T,
                        np.sin(freqs + d[h][None, :]).T,
                    ],
                    axis=0,
                ).astype(np.float32)
                for h in heads
            ],
            axis=0,
        )  # (4, 128, T)

        in_maps.append(
            {
                "xT": xT,
                "wqk": wqk,
                "wv": wv,
                "w2t": w2t,
                "trig": trig,
                "ab": ab,
                "cmask": cmask,
            }
        )
    return in_maps


_NC_CACHE = []


def _get_nc():
    if not _NC_CACHE:
        _NC_CACHE.append(build_module())
    return _NC_CACHE[0]


def kernel(x, w_attn, w_proj, delta, _trace=False):
    in_maps = make_inputs(x, w_attn, w_proj, delta)
    nc = _get_nc()
    res = None
    outs = None
    last_err = None
    for attempt in range(3):
        try:
            res = bass_utils.run_bass_kernel_spmd(
                nc, in_maps, core_ids=list(range(N_CORES)), trace=_trace
            )
            # jax results are async: force materialization inside the
            # retry so a transient NRT_EXEC_UNIT_UNRECOVERABLE (seen on
            # the first execution of a freshly-loaded NEFF) is caught
            outs = [
                np.asarray(r["out"]).reshape(T, C) for r in res.results
            ]
            break
        except Exception as e:
            last_err = e
            if "unrecoverable" not in str(e).lower() or attempt == 2:
                raise
            import time as _time

            _time.sleep(2.0)
    assert outs is not None, last_err
    if _trace:
        kernel.last_results = res
    full = np.zeros((B, T, C), dtype=np.float32)
    for core in range(N_CORES):
        full[core // HPC] += outs[core]
    return full



# revision 44
# speedup vs baseline: 1.2495x; 1.2495x over previous
"""Trainium2 Bass kernel for causal self-attention with PoPE (v2).

Reference computation (B=2, T=2048, C=1024, H=16, D=64):
  qkv = x @ w_attn.T ; split q,k,v ; heads
  mu_q = softplus(q); mu_k = softplus(k)
  q_real = mu_q * cos(t w); q_imag = mu_q * sin(t w)
  k_real = mu_k * cos(t w + d); k_imag = mu_k * sin(t w + d)
  att = softmax_causal((q_real k_real + q_imag k_imag)/sqrt(D))
  y = att @ v ; out = y @ w_proj.T

Sharding: 8 cores = 2 batches x 4 head-groups (4 heads each). Each core
computes its batch's QKV for its heads, attention, and a partial c_proj
(its heads' input-channel rows of w_proj). Host sums the 4 partials per
batch.

v2 design (vs the previous 172.7us version):
  * bf16 activations (x, trig, mu, P, y) with f32r weights: halves DMA
    bytes, keeps every matmul at 1 cycle/row (cost keys on the moving
    operand), and doubles DVE throughput on the elementwise path.
  * y matmul flipped to out[tq,d] (free dim 65 instead of 512): halves
    its PE cost AND lands the softmax denominator as a per-partition
    scalar (psum col 64), so normalization is a [128,4] reciprocal + 4
    tensor_scalar ops instead of reciprocal+PE-broadcast+mul. The sweep
    runs b-outer so only one psum accumulation group is open per bank.
  * y^T for c_proj via PE transpose (2 heads x 1 tq-128 block per op).
  * c_proj fused per tq-128 block right after the last head's
    transpose: output DMAs spread across the whole attention phase.
  * Phase A softplus batches all Exps then all Lns (the act-table pass
    otherwise reloads tables on every Exp<->Ln switch).
  * every psum pool is opened once at the top (QK shares the S pool's
    tag) - no pool-boundary chaining stalls at phase transitions.
  * the (j,h) y-sweep is deferred behind the next score pass's first
    S pair so ACT (the bottleneck engine, ~95us busy) never starves.
"""

import math
import os
import sys

import numpy as np

for _p in ("/opt/trn_rl_repo",):
    if _p not in sys.path and os.path.isdir(_p):
        sys.path.insert(0, _p)

import ml_dtypes

import concourse.tile as tile
from concourse import bacc
from concourse import mybir
from concourse import bass_utils

B, T, C = 2, 2048, 1024
H, D = 16, 64
BASE = 10000.0
N_CORES = 8
HPC = 4  # heads per core
NCT = 8  # c tiles (1024/128)
NTT = 16  # t tiles of 128

F32 = mybir.dt.float32
F32R = mybir.dt.float32r
BF16 = mybir.dt.bfloat16
AF = mybir.ActivationFunctionType


def build_module():
    nc = bacc.Bacc(
        "TRN2", target_bir_lowering=False, debug=False, num_devices=N_CORES
    )

    xT_d = nc.dram_tensor("xT", (NCT, 128, T), BF16, kind="ExternalInput").ap()
    wqk_d = nc.dram_tensor("wqk", (NCT, 128, 512), BF16, kind="ExternalInput").ap()
    wv_d = nc.dram_tensor("wv", (NCT, 128, 256), BF16, kind="ExternalInput").ap()
    w2_d = nc.dram_tensor("w2", (2, 128, 1024), BF16, kind="ExternalInput").ap()
    trig_d = nc.dram_tensor("trig", (128, T), BF16, kind="ExternalInput").ap()
    ab_d = nc.dram_tensor("ab", (HPC, 128, T), BF16, kind="ExternalInput").ap()
    cmask_d = nc.dram_tensor("cmask", (128, 128), BF16, kind="ExternalInput").ap()
    ident_d = nc.dram_tensor("ident", (128, 128), BF16, kind="ExternalInput").ap()
    out_d = nc.dram_tensor("out", (NTT, 128, 1024), F32, kind="ExternalOutput").ap()

    idm = list(range(32))

    with tile.TileContext(nc) as tc:
        with (
            tc.tile_pool(name="persist", bufs=1) as persist,
            tc.tile_pool(name="shufp", bufs=2) as shufp,
            tc.tile_pool(name="pp", bufs=9) as pp,
            tc.tile_pool(name="rcp", bufs=2) as rcp,
            tc.tile_pool(name="ynp", bufs=2) as ynp,
            tc.tile_pool(name="ostp", bufs=4) as ostp,
            tc.tile_pool(name="ps_s", bufs=2, space="PSUM") as ps_s,
            tc.tile_pool(name="ps_y", bufs=1, space="PSUM") as ps_y,
            tc.tile_pool(name="ps_t", bufs=1, space="PSUM") as ps_t,
            tc.tile_pool(name="ps_o", bufs=2, space="PSUM") as ps_o,
        ):
            # ---------------- persistent tiles ----------------
            v_aug = persist.tile([128, NTT, HPC, 65], BF16)
            nc.vector.memset(v_aug[:, :, :, 64:65], 1.0)
            trig = persist.tile([128, T], BF16)
            ab = persist.tile([128, HPC, T], BF16)
            cmask = persist.tile([128, 128], BF16)
            ident = persist.tile([128, 128], BF16)
            bias_m1 = persist.tile([128, 1], F32)
            nc.vector.memset(bias_m1, -1.0)
            mu = [persist.tile([128, T], BF16, name=f"mu{h}") for h in range(HPC)]
            qt = [persist.tile([128, T], BF16, name=f"qt{h}") for h in range(HPC)]
            kt = [persist.tile([128, T], BF16, name=f"kt{h}") for h in range(HPC)]
            y_t = persist.tile([128, 2, T], BF16)
            w2 = persist.tile([128, 2, 1024], BF16)
            wqk = persist.tile([128, NCT, 512], BF16)
            wv = persist.tile([128, NCT, 256], BF16)
            xT = persist.tile([128, NCT, T], BF16)

            # ------------- input DMAs (chunked; ACT only at idle start) ----
            xt_eng = {0: nc.sync, 3: nc.sync, 6: nc.sync,
                      1: nc.gpsimd, 4: nc.gpsimd, 7: nc.gpsimd,
                      2: nc.scalar, 5: nc.scalar}
            for c in range(NCT):
                nc.sync.dma_start(wqk[:, c, :], wqk_d[c])
                xt_eng[c].dma_start(xT[:, c, :], xT_d[c])
            nc.gpsimd.dma_start(wv, wv_d.rearrange("o p e -> p o e"))
            # trig/ab on the ACT queue: it is idle until the first QK psum
            # completes (~14us), so these transfers are free there
            nc.scalar.dma_start(trig, trig_d)
            for h in range(HPC):
                nc.scalar.dma_start(ab[:, h, :], ab_d[h])
            nc.gpsimd.dma_start(cmask, cmask_d)
            nc.sync.dma_start(ident, ident_d)
            nc.sync.dma_start(w2, w2_d.rearrange("o p e -> p o e"))

            # ---------------- Phase A: QKV projection ----------------
            # exp/ln batched per head-pair, with explicit ACT ordering so
            # the scheduler can't interleave Exp<->Ln (each switch costs a
            # 1283ns act-table reload): 5 loads total instead of 15.
            from concourse.tile_rust import add_dep_helper

            last_act = [None]

            def ordered_act(handle):
                if last_act[0] is not None:
                    add_dep_helper(
                        handle.ins, last_act[0].ins, sync=False,
                        reason="batch ACT ops to minimize table reloads",
                    )
                last_act[0] = handle

            def v_chunk(tts):
                # V tiles borrow the ps_o pool (temporally disjoint from
                # c_proj use: V at chunk start, c_proj at end)
                for tt in tts:
                    psv = ps_o.tile([128, 512], F32, tag="o", name="ps_o")
                    for c in range(NCT):
                        nc.tensor.matmul(
                            psv[:, 0:256],
                            lhsT=xT[:, c, tt * 128 : (tt + 1) * 128],
                            rhs=wv[:, c, :],
                            start=(c == 0),
                            stop=(c == NCT - 1),
                        )
                    nc.vector.tensor_copy(
                        out=v_aug[:, tt, :, 0:64],
                        in_=psv[:, 0:256].rearrange("p (h e) -> p h e", h=HPC),
                    )

            def phaseA_pair(hp):
                for h in (2 * hp, 2 * hp + 1):
                    for tbp in range(2):
                        ps = ps_s.tile([128, 1024], F32, tag="s", name="ps_s")
                        base = tbp * 1024
                        for c in range(NCT):
                            for half in range(2):
                                o0 = half * 512
                                nc.tensor.matmul(
                                    ps[:, o0 : o0 + 512],
                                    lhsT=wqk[:, c, h * 128 : (h + 1) * 128],
                                    rhs=xT[:, c, base + o0 : base + o0 + 512],
                                    start=(c == 0),
                                    stop=(c == NCT - 1),
                                )
                        ordered_act(
                            nc.scalar.activation(
                                mu[h][:, base : base + 1024], ps, AF.Exp
                            )
                        )
                for h in (2 * hp, 2 * hp + 1):
                    ordered_act(
                        nc.scalar.activation(
                            mu[h], mu[h], AF.Ln, bias=1.0
                        )
                    )
                    # per-head preprocessing (overlaps later QK/V matmuls):
                    # qt rows 0:64 = mu_q cos(tw)/8, 64:128 = mu_q sin(tw)/8
                    # kt rows 0:64 = mu_k cos(tw+d), 64:128 = mu_k sin(tw+d)
                    # partition swap via SBUF->SBUF DMA on the idle SP queue
                    # (vs 2194ns of precious DVE per stream_shuffle), halved
                    # and interleaved so the first S matmuls start ~2us
                    # after the ln instead of ~5us
                    mks = shufp.tile([128, T], BF16, tag="mks", name="mks")
                    for th in range(2):
                        ts_ = slice(th * 1024, th * 1024 + 1024)
                        nc.sync.dma_start(mks[0:64, ts_], mu[h][64:128, ts_])
                        nc.sync.dma_start(mks[64:128, ts_], mu[h][0:64, ts_])
                        # mks-independent halves first (no DMA wait)
                        nc.gpsimd.tensor_mul(
                            qt[h][0:64, ts_], mu[h][0:64, ts_], trig[0:64, ts_]
                        )
                        nc.vector.tensor_mul(
                            kt[h][64:128, ts_], mu[h][64:128, ts_],
                            ab[64:128, h, ts_],
                        )
                        nc.vector.tensor_mul(
                            kt[h][0:64, ts_], mks[0:64, ts_], ab[0:64, h, ts_]
                        )
                        nc.gpsimd.tensor_mul(
                            qt[h][64:128, ts_], mks[64:128, ts_],
                            trig[64:128, ts_],
                        )

            # Phase A pairs with the j0 V chunks as PE filler while the
            # first pair's lns + preprocessing drain on ACT/DVE
            phaseA_pair(0)
            v_chunk(range(0, 4))
            phaseA_pair(1)

            # ------------- Phase B: attention + fused c_proj -------------
            HPCR = (0, 1, 2, 3)
            ypn_ref = [None]

            def c_proj_tt(tt, last=False):
                for eh in range(2):
                    po = ps_o.tile([128, 512], F32, tag="o", name="ps_o")
                    for ct in range(2):
                        nc.tensor.matmul(
                            po,
                            lhsT=y_t[:, ct, tt * 128 : (tt + 1) * 128],
                            rhs=w2[:, ct, eh * 512 : eh * 512 + 512],
                            start=(ct == 0),
                            stop=(ct == 1),
                        )
                    ost = ostp.tile([128, 512], F32, tag="ost", name="ost")
                    if last and eh == 1:
                        # final chunk: ACT is drained by now - use it for
                        # the copy (Copy is in every act table) + DMA so
                        # the tail isn't serialized on SP/Pool
                        nc.scalar.activation(ost, po, AF.Copy)
                        nc.scalar.dma_start(
                            out_d[tt][:, eh * 512 : eh * 512 + 512], ost
                        )
                    elif eh == 0:
                        nc.vector.tensor_copy(out=ost, in_=po)
                        nc.sync.dma_start(
                            out_d[tt][:, eh * 512 : eh * 512 + 512], ost
                        )
                    else:
                        nc.vector.tensor_copy(out=ost, in_=po)
                        nc.gpsimd.dma_start(
                            out_d[tt][:, eh * 512 : eh * 512 + 512], ost
                        )

            def flush_stages(pend):
                """Stages of the y-sweep + normalize (+transpose/c_proj)
                for a completed (j, h) score pass, to be interleaved one
                per S-pair of the next pass so the PE queue never sees a
                long non-score burst. b-outer keeps at most one pending
                psum accumulation group per bank."""
                jf, hf, p_map = pend
                yp = ps_y.tile([128, 4, 65], F32, tag="y", name="ps_y")

                def sweep(b_):
                    for i in range(4 * jf + b_ + 1):
                        tile_, sub = p_map[i]
                        nc.tensor.matmul(
                            yp[:, b_, :],
                            lhsT=tile_[
                                :, sub * 512 + b_ * 128 : sub * 512 + b_ * 128 + 128
                            ],
                            rhs=v_aug[:, i, hf, :],
                            start=(i == 0),
                            stop=(i == 4 * jf + b_),
                        )

                def normalize():
                    rc = rcp.tile([128, 4], F32, tag="rc", name="rc")
                    with nc.allow_low_precision(
                        reason="f32 reciprocal of softmax denominator"
                    ):
                        nc.vector.reciprocal(rc, yp[:, :, 64])
                    if hf % 2 == 0:
                        ypn_ref[0] = ynp.tile(
                            [128, 4, 2, 64], BF16, tag="yn", name="ypn"
                        )
                    ypn = ypn_ref[0]
                    for b_ in range(4):
                        nc.vector.tensor_scalar_mul(
                            ypn[:, b_, hf % 2, :], yp[:, b_, 0:64], rc[:, b_ : b_ + 1]
                        )

                def finalize():
                    if hf % 2 == 0:
                        return
                    # transpose 2 heads x [tq-128, 64] -> [128, tq-128]
                    hp = hf // 2
                    ypn = ypn_ref[0]
                    tp = ps_t.tile([128, 512], BF16, tag="t", name="ps_t")
                    for b_ in range(4):
                        nc.tensor.transpose(
                            tp[:, b_ * 128 : (b_ + 1) * 128],
                            ypn[:, b_].rearrange("p a d -> p (a d)"),
                            ident,
                        )
                    if hf == 3:
                        # per-b copy so c_proj(tt) streams out early
                        for b_ in range(4):
                            nc.vector.tensor_copy(
                                out=y_t[
                                    :, hp, jf * 512 + b_ * 128 : jf * 512 + b_ * 128 + 128
                                ],
                                in_=tp[:, b_ * 128 : (b_ + 1) * 128],
                            )
                            c_proj_tt(4 * jf + b_, last=(jf == 0))
                    else:
                        nc.vector.tensor_copy(
                            out=y_t[:, hp, jf * 512 : (jf + 1) * 512], in_=tp
                        )

                return [
                    lambda: sweep(0),
                    lambda: sweep(1),
                    lambda: sweep(2),
                    lambda: (sweep(3), normalize()),
                    finalize,
                ]

            pending = []

            def score_pass(j, h):
                nblk = 4 * j + 4
                # pair each diagonal block (as sub0, so its valid range
                # [c0:512] abuts sub1) with a non-diagonal block: the exp
                # range [c0:1024] stays contiguous -> one ACT op per tile
                diag = list(range(4 * j, nblk))
                nond = list(range(0, 4 * j))
                if nond:
                    pairs = list(zip(diag, nond[: len(diag)]))
                    rest = nond[len(diag) :]
                    pairs += [(rest[k], rest[k + 1]) for k in range(0, len(rest), 2)]
                else:
                    pairs = [(0, 1), (2, 3)]
                p_map = {}
                for i0, i1 in pairs:
                    sp = ps_s.tile([128, 1024], F32, tag="s", name="ps_s")
                    p_sb = pp.tile([128, 1024], BF16, tag="p", name="p_sb")
                    for sub, i in ((0, i0), (1, i1)):
                        c0 = 128 * max(0, i - 4 * j)
                        nc.tensor.matmul(
                            sp[:, sub * 512 + c0 : sub * 512 + 512],
                            lhsT=kt[h][:, i * 128 : (i + 1) * 128],
                            rhs=qt[h][:, j * 512 + c0 : (j + 1) * 512],
                            start=True,
                            stop=True,
                        )
                        p_map[i] = (p_sb, sub)
                    c00 = 128 * max(0, i0 - 4 * j)
                    c01 = 128 * max(0, i1 - 4 * j)
                    if c01 == 0:
                        nc.scalar.activation(
                            p_sb[:, c00:1024], sp[:, c00:1024], AF.Exp, bias=bias_m1
                        )
                    else:  # j=0: both blocks diagonal, 2 ops
                        nc.scalar.activation(
                            p_sb[:, c00:512], sp[:, c00:512], AF.Exp, bias=bias_m1
                        )
                        nc.scalar.activation(
                            p_sb[:, 512 + c01 : 1024],
                            sp[:, 512 + c01 : 1024],
                            AF.Exp,
                            bias=bias_m1,
                        )
                    # mask diagonal strips (Pool)
                    for sub, i in ((0, i0), (1, i1)):
                        if 4 * j <= i <= 4 * j + 3:
                            boff = sub * 512 + 128 * (i - 4 * j)
                            nc.gpsimd.tensor_mul(
                                p_sb[:, boff : boff + 128],
                                p_sb[:, boff : boff + 128],
                                cmask,
                            )
                    # consume one flush stage of the previous (j, h)
                    # behind each S pair: the PE stream alternates score
                    # matmuls with y-sweep chunks and ACT stays fed
                    if pending:
                        pending.pop(0)()
                # carry at most the finalize stage into the next pass (the
                # P tiles are released once sweep3 ran, keeping pp bounded)
                while len(pending) > 1:
                    pending.pop(0)()
                pending.extend(flush_stages((j, h, p_map)))

            v_chunk(range(4, 8))
            for j, hs in ((1, HPCR), (2, HPCR), (3, HPCR), (0, HPCR)):
                if j in (2, 3):
                    v_chunk(range(4 * j, 4 * j + 4))
                for h in hs:
                    score_pass(j, h)
            while pending:
                pending.pop(0)()

    nc.compile()
    return nc


def make_inputs(x, w_attn, w_proj, delta):
    """Host-side prep: per-core input dicts (core = b*4 + g)."""
    x = np.asarray(x, dtype=np.float32)
    w_attn = np.asarray(w_attn, dtype=np.float32)
    w_proj = np.asarray(w_proj, dtype=np.float32)
    delta = np.asarray(delta, dtype=np.float32)
    bf = ml_dtypes.bfloat16

    inv_freq = 1.0 / (BASE ** (np.arange(D, dtype=np.float32) / D))
    t = np.arange(T, dtype=np.float32)
    freqs = t[:, None] * inv_freq[None, :]  # (T, D)
    scale = 1.0 / math.sqrt(D)
    trig = np.concatenate(
        [np.cos(freqs).T * scale, np.sin(freqs).T * scale], axis=0
    ).astype(bf)  # (128, T)

    d = np.clip(delta, -2.0 * math.pi, 0.0)

    qw = w_attn[:C].reshape(H, D, C)
    kw = w_attn[C : 2 * C].reshape(H, D, C)
    vw = w_attn[2 * C :].reshape(H, D, C)

    # causal mask for diagonal 128-blocks of P^T [tk, tq]: valid iff tq >= tk
    tk = np.arange(128)[:, None]
    cc = np.arange(128)[None, :]
    cmask = (cc >= tk).astype(bf)
    ident = np.eye(128, dtype=np.float32).astype(bf)

    in_maps = []
    for core in range(N_CORES):
        b, g = divmod(core, HPC)
        heads = range(HPC * g, HPC * g + HPC)

        xT = np.ascontiguousarray(x[b].T).reshape(NCT, 128, T).astype(bf)

        qk = np.stack(
            [np.concatenate([qw[h], kw[h]], axis=0) for h in heads], axis=0
        )  # (4, 128, C)
        wqk = np.ascontiguousarray(qk.transpose(2, 0, 1).reshape(C, 512)).reshape(
            NCT, 128, 512
        ).astype(bf)
        wv = np.ascontiguousarray(
            vw[HPC * g : HPC * g + HPC].reshape(256, C).T
        ).reshape(NCT, 128, 256).astype(bf)
        w2t = np.ascontiguousarray(
            w_proj[:, 256 * g : 256 * (g + 1)].T
        ).reshape(2, 128, 1024).astype(bf)

        ab = np.stack(
            [
                np.concatenate(
                    [
                        np.cos(freqs + d[h][None, :]).T,
                        np.sin(freqs + d[h][None, :]).T,
                    ],
                    axis=0,
                )
                for h in heads
            ],
            axis=0,
        ).astype(bf)  # (4, 128, T)

        in_maps.append(
            {
                "xT": xT,
                "wqk": wqk,
                "wv": wv,
                "w2": w2t,
                "trig": trig,
                "ab": ab,
                "cmask": cmask,
                "ident": ident,
            }
        )
    return in_maps


_NC_CACHE = []


def _get_nc():
    if not _NC_CACHE:
        _NC_CACHE.append(build_module())
    return _NC_CACHE[0]


def kernel(x, w_attn, w_proj, delta, _trace=False):
    in_maps = make_inputs(x, w_attn, w_proj, delta)
    nc = _get_nc()
    res = None
    outs = None
    last_err = None
    for attempt in range(3):
        try:
            res = bass_utils.run_bass_kernel_spmd(
                nc, in_maps, core_ids=list(range(N_CORES)), trace=_trace
            )
            outs = [
                np.asarray(r["out"]).reshape(T, C) for r in res.results
            ]
            break
        except Exception as e:
            last_err = e
            if "unrecoverable" not in str(e).lower() or attempt == 2:
                raise
            import time as _time

            _time.sleep(2.0)
    assert outs is not None, last_err
    if _trace:
        kernel.last_results = res
    full = np.zeros((B, T, C), dtype=np.float32)
    for core in range(N_CORES):
        full[core // HPC] += outs[core]
    return full


# revision 56
# speedup vs baseline: 1.2578x; 1.0067x over previous
"""Trainium2 Bass kernel for causal self-attention with PoPE (v2).

Reference computation (B=2, T=2048, C=1024, H=16, D=64):
  qkv = x @ w_attn.T ; split q,k,v ; heads
  mu_q = softplus(q); mu_k = softplus(k)
  q_real = mu_q * cos(t w); q_imag = mu_q * sin(t w)
  k_real = mu_k * cos(t w + d); k_imag = mu_k * sin(t w + d)
  att = softmax_causal((q_real k_real + q_imag k_imag)/sqrt(D))
  y = att @ v ; out = y @ w_proj.T

Sharding: 8 cores = 2 batches x 4 head-groups (4 heads each). Each core
computes its batch's QKV for its heads, attention, and a partial c_proj
(its heads' input-channel rows of w_proj). Host sums the 4 partials per
batch.

v2 design (vs the previous 172.7us version):
  * bf16 activations (x, trig, mu, P, y) with f32r weights: halves DMA
    bytes, keeps every matmul at 1 cycle/row (cost keys on the moving
    operand), and doubles DVE throughput on the elementwise path.
  * y matmul flipped to out[tq,d] (free dim 65 instead of 512): halves
    its PE cost AND lands the softmax denominator as a per-partition
    scalar (psum col 64), so normalization is a [128,4] reciprocal + 4
    tensor_scalar ops instead of reciprocal+PE-broadcast+mul. The sweep
    runs b-outer so only one psum accumulation group is open per bank.
  * y^T for c_proj via PE transpose (2 heads x 1 tq-128 block per op).
  * c_proj fused per tq-128 block right after the last head's
    transpose: output DMAs spread across the whole attention phase.
  * Phase A softplus batches all Exps then all Lns (the act-table pass
    otherwise reloads tables on every Exp<->Ln switch).
  * every psum pool is opened once at the top (QK shares the S pool's
    tag) - no pool-boundary chaining stalls at phase transitions.
  * the (j,h) y-sweep is deferred behind the next score pass's first
    S pair so ACT (the bottleneck engine, ~95us busy) never starves.
"""

import math
import os
import sys

import numpy as np

for _p in ("/opt/trn_rl_repo",):
    if _p not in sys.path and os.path.isdir(_p):
        sys.path.insert(0, _p)

import ml_dtypes

import concourse.tile as tile
from concourse import bacc
from concourse import mybir
from concourse import bass_utils

B, T, C = 2, 2048, 1024
H, D = 16, 64
BASE = 10000.0
N_CORES = 8
HPC = 4  # heads per core
NCT = 8  # c tiles (1024/128)
NTT = 16  # t tiles of 128

F32 = mybir.dt.float32
F32R = mybir.dt.float32r
BF16 = mybir.dt.bfloat16
AF = mybir.ActivationFunctionType


def build_module():
    nc = bacc.Bacc(
        "TRN2", target_bir_lowering=False, debug=False, num_devices=N_CORES
    )

    xT_d = nc.dram_tensor("xT", (NCT, 128, T), BF16, kind="ExternalInput").ap()
    wqk_d = nc.dram_tensor("wqk", (NCT, 128, 512), BF16, kind="ExternalInput").ap()
    wv_d = nc.dram_tensor("wv", (NCT, 128, 256), BF16, kind="ExternalInput").ap()
    w2_d = nc.dram_tensor("w2", (2, 128, 1024), BF16, kind="ExternalInput").ap()
    trig_d = nc.dram_tensor("trig", (128, T), BF16, kind="ExternalInput").ap()
    ab_d = nc.dram_tensor("ab", (HPC, 128, T), BF16, kind="ExternalInput").ap()
    cmask_d = nc.dram_tensor("cmask", (128, 128), BF16, kind="ExternalInput").ap()
    ident_d = nc.dram_tensor("ident", (128, 128), BF16, kind="ExternalInput").ap()
    out_d = nc.dram_tensor("out", (NTT, 128, 1024), F32, kind="ExternalOutput").ap()

    idm = list(range(32))

    with tile.TileContext(nc) as tc:
        with (
            tc.tile_pool(name="persist", bufs=1) as persist,
            tc.tile_pool(name="shufp", bufs=2) as shufp,
            tc.tile_pool(name="pp", bufs=9) as pp,
            tc.tile_pool(name="rcp", bufs=2) as rcp,
            tc.tile_pool(name="ynp", bufs=2) as ynp,
            tc.tile_pool(name="ostp", bufs=4) as ostp,
            tc.tile_pool(name="ps_s", bufs=2, space="PSUM") as ps_s,
            tc.tile_pool(name="ps_y", bufs=1, space="PSUM") as ps_y,
            tc.tile_pool(name="ps_t", bufs=1, space="PSUM") as ps_t,
            tc.tile_pool(name="ps_o", bufs=2, space="PSUM") as ps_o,
        ):
            # ---------------- persistent tiles ----------------
            v_aug = persist.tile([128, NTT, HPC, 65], BF16)
            nc.vector.memset(v_aug[:, :, :, 64:65], 1.0)
            trig = persist.tile([128, T], BF16)
            ab = persist.tile([128, HPC, T], BF16)
            cmask = persist.tile([128, 128], BF16)
            ident = persist.tile([128, 128], BF16)
            bias_m1 = persist.tile([128, 1], F32)
            nc.vector.memset(bias_m1, -1.0)
            mu = [persist.tile([128, T], BF16, name=f"mu{h}") for h in range(HPC)]
            qt = [persist.tile([128, T], BF16, name=f"qt{h}") for h in range(HPC)]
            kt = [persist.tile([128, T], BF16, name=f"kt{h}") for h in range(HPC)]
            y_t = persist.tile([128, 2, T], BF16)
            w2 = persist.tile([128, 2, 1024], BF16)
            wqk = persist.tile([128, NCT, 512], BF16)
            wv = persist.tile([128, NCT, 256], BF16)
            xT = persist.tile([128, NCT, T], BF16)

            # ------------- input DMAs (chunked; ACT only at idle start) ----
            xt_eng = {0: nc.sync, 3: nc.sync, 6: nc.sync,
                      1: nc.gpsimd, 4: nc.gpsimd, 7: nc.gpsimd,
                      2: nc.scalar, 5: nc.scalar}
            for c in range(NCT):
                weng = nc.gpsimd if c >= 6 else nc.sync
                weng.dma_start(wqk[:, c, :], wqk_d[c])
                xt_eng[c].dma_start(xT[:, c, :], xT_d[c])
            nc.gpsimd.dma_start(wv, wv_d.rearrange("o p e -> p o e"))
            # trig/ab0-1 on the ACT queue: it is idle until the first QK
            # psum completes, so these transfers are free there; ab2-3
            # ride late on SP (needed only at prep h2/h3)
            nc.scalar.dma_start(trig, trig_d)
            for h in range(2):
                nc.scalar.dma_start(ab[:, h, :], ab_d[h])
            nc.gpsimd.dma_start(cmask, cmask_d)
            nc.sync.dma_start(ident, ident_d)
            nc.sync.dma_start(w2, w2_d.rearrange("o p e -> p o e"))
            for h in range(2, HPC):
                nc.sync.dma_start(ab[:, h, :], ab_d[h])

            # ---------------- Phase A: QKV projection ----------------
            # exp/ln batched per head-pair, with explicit ACT ordering so
            # the scheduler can't interleave Exp<->Ln (each switch costs a
            # 1283ns act-table reload): 5 loads total instead of 15.
            from concourse.tile_rust import add_dep_helper

            last_act = [None]

            def ordered_act(handle):
                if last_act[0] is not None:
                    add_dep_helper(
                        handle.ins, last_act[0].ins, sync=False,
                        reason="batch ACT ops to minimize table reloads",
                    )
                last_act[0] = handle

            def v_chunk(tts):
                # V tiles borrow the ps_o pool (temporally disjoint from
                # c_proj use: V at chunk start, c_proj at end)
                for tt in tts:
                    psv = ps_o.tile([128, 512], F32, tag="o", name="ps_o")
                    for c in range(NCT):
                        nc.tensor.matmul(
                            psv[:, 0:256],
                            lhsT=xT[:, c, tt * 128 : (tt + 1) * 128],
                            rhs=wv[:, c, :],
                            start=(c == 0),
                            stop=(c == NCT - 1),
                        )
                    nc.vector.tensor_copy(
                        out=v_aug[:, tt, :, 0:64],
                        in_=psv[:, 0:256].rearrange("p (h e) -> p h e", h=HPC),
                    )

            def phaseA_pair(hp, interleave=()):
                k = 0
                for h in (2 * hp, 2 * hp + 1):
                    for tbp in range(2):
                        ps = ps_s.tile([128, 1024], F32, tag="s", name="ps_s")
                        base = tbp * 1024
                        for c in range(NCT):
                            for half in range(2):
                                o0 = half * 512
                                nc.tensor.matmul(
                                    ps[:, o0 : o0 + 512],
                                    lhsT=wqk[:, c, h * 128 : (h + 1) * 128],
                                    rhs=xT[:, c, base + o0 : base + o0 + 512],
                                    start=(c == 0),
                                    stop=(c == NCT - 1),
                                )
                        ordered_act(
                            nc.scalar.activation(
                                mu[h][:, base : base + 1024], ps, AF.Exp
                            )
                        )
                        if k < len(interleave):
                            interleave[k]()
                            k += 1
                for h in (2 * hp, 2 * hp + 1):
                    ordered_act(
                        nc.scalar.activation(
                            mu[h], mu[h], AF.Ln, bias=1.0
                        )
                    )
                    # per-head preprocessing (overlaps later QK/V matmuls):
                    # qt rows 0:64 = mu_q cos(tw)/8, 64:128 = mu_q sin(tw)/8
                    # kt rows 0:64 = mu_k cos(tw+d), 64:128 = mu_k sin(tw+d)
                    # partition swap via SBUF->SBUF DMA on the idle SP queue
                    # (vs 2194ns of precious DVE per stream_shuffle), halved
                    # and interleaved so the first S matmuls start ~2us
                    # after the ln instead of ~5us
                    mks = shufp.tile([128, T], BF16, tag="mks", name="mks")
                    for th in range(2):
                        ts_ = slice(th * 1024, th * 1024 + 1024)
                        nc.sync.dma_start(mks[0:64, ts_], mu[h][64:128, ts_])
                        nc.sync.dma_start(mks[64:128, ts_], mu[h][0:64, ts_])
                        # mks-independent halves first (no DMA wait)
                        nc.gpsimd.tensor_mul(
                            qt[h][0:64, ts_], mu[h][0:64, ts_], trig[0:64, ts_]
                        )
                        nc.vector.tensor_mul(
                            kt[h][64:128, ts_], mu[h][64:128, ts_],
                            ab[64:128, h, ts_],
                        )
                        nc.vector.tensor_mul(
                            kt[h][0:64, ts_], mks[0:64, ts_], ab[0:64, h, ts_]
                        )
                        nc.gpsimd.tensor_mul(
                            qt[h][64:128, ts_], mks[64:128, ts_],
                            trig[64:128, ts_],
                        )

            # Phase A pairs with the j0 V chunks as PE filler while the
            # first pair's lns + preprocessing drain on ACT/DVE
            phaseA_pair(0)
            v_chunk(range(0, 4))

            # ------------- Phase B: attention + fused c_proj -------------
            HPCR = (0, 1, 2, 3)
            ypn_ref = [None]

            def c_proj_tt(tt, last=False):
                for eh in range(2):
                    po = ps_o.tile([128, 512], F32, tag="o", name="ps_o")
                    for ct in range(2):
                        nc.tensor.matmul(
                            po,
                            lhsT=y_t[:, ct, tt * 128 : (tt + 1) * 128],
                            rhs=w2[:, ct, eh * 512 : eh * 512 + 512],
                            start=(ct == 0),
                            stop=(ct == 1),
                        )
                    ost = ostp.tile([128, 512], F32, tag="ost", name="ost")
                    if last and eh == 1:
                        # final chunk: ACT is drained by now - use it for
                        # the copy (Copy is in every act table) + DMA so
                        # the tail isn't serialized on SP/Pool
                        nc.scalar.activation(ost, po, AF.Copy)
                        nc.scalar.dma_start(
                            out_d[tt][:, eh * 512 : eh * 512 + 512], ost
                        )
                    elif eh == 0:
                        nc.vector.tensor_copy(out=ost, in_=po)
                        nc.sync.dma_start(
                            out_d[tt][:, eh * 512 : eh * 512 + 512], ost
                        )
                    else:
                        nc.vector.tensor_copy(out=ost, in_=po)
                        nc.gpsimd.dma_start(
                            out_d[tt][:, eh * 512 : eh * 512 + 512], ost
                        )

            def flush_stages(pend):
                """Stages of the y-sweep + normalize (+transpose/c_proj)
                for a completed (j, h) score pass, to be interleaved one
                per S-pair of the next pass so the PE queue never sees a
                long non-score burst. b-outer keeps at most one pending
                psum accumulation group per bank."""
                jf, hf, p_map = pend
                yp = ps_y.tile([128, 4, 65], F32, tag="y", name="ps_y")

                def sweep(b_):
                    for i in range(4 * jf + b_ + 1):
                        tile_, sub = p_map[i]
                        nc.tensor.matmul(
                            yp[:, b_, :],
                            lhsT=tile_[
                                :, sub * 512 + b_ * 128 : sub * 512 + b_ * 128 + 128
                            ],
                            rhs=v_aug[:, i, hf, :],
                            start=(i == 0),
                            stop=(i == 4 * jf + b_),
                        )

                def normalize():
                    rc = rcp.tile([128, 4], F32, tag="rc", name="rc")
                    with nc.allow_low_precision(
                        reason="f32 reciprocal of softmax denominator"
                    ):
                        nc.vector.reciprocal(rc, yp[:, :, 64])
                    if hf % 2 == 0:
                        ypn_ref[0] = ynp.tile(
                            [128, 4, 2, 64], BF16, tag="yn", name="ypn"
                        )
                    ypn = ypn_ref[0]
                    for b_ in range(4):
                        nc.vector.tensor_scalar_mul(
                            ypn[:, b_, hf % 2, :], yp[:, b_, 0:64], rc[:, b_ : b_ + 1]
                        )

                def finalize():
                    if hf % 2 == 0:
                        return
                    # transpose 2 heads x [tq-128, 64] -> [128, tq-128]
                    hp = hf // 2
                    ypn = ypn_ref[0]
                    tp = ps_t.tile([128, 512], BF16, tag="t", name="ps_t")
                    for b_ in range(4):
                        nc.tensor.transpose(
                            tp[:, b_ * 128 : (b_ + 1) * 128],
                            ypn[:, b_].rearrange("p a d -> p (a d)"),
                            ident,
                        )
                    if hf == 3:
                        # per-b copy so c_proj(tt) streams out early
                        for b_ in range(4):
                            nc.vector.tensor_copy(
                                out=y_t[
                                    :, hp, jf * 512 + b_ * 128 : jf * 512 + b_ * 128 + 128
                                ],
                                in_=tp[:, b_ * 128 : (b_ + 1) * 128],
                            )
                            c_proj_tt(4 * jf + b_, last=(jf == 0))
                    else:
                        nc.vector.tensor_copy(
                            out=y_t[:, hp, jf * 512 : (jf + 1) * 512], in_=tp
                        )

                if jf == 0 and hf == 3:
                    # very last pass: per-b chains, largest b first, so
                    # the kernel tail is only b=0's short chain
                    def tail_b(b_):
                        sweep(b_)
                        rc = rcp.tile([128, 1], F32, tag="rc1", name="rc1")
                        with nc.allow_low_precision(
                            reason="f32 reciprocal of softmax denominator"
                        ):
                            nc.vector.reciprocal(rc, yp[:, b_, 64:65])
                        ypn = ypn_ref[0]
                        nc.vector.tensor_scalar_mul(
                            ypn[:, b_, 1, :], yp[:, b_, 0:64], rc
                        )
                        tp = ps_t.tile([128, 512], BF16, tag="t", name="ps_t")
                        nc.tensor.transpose(
                            tp[:, 0:128],
                            ypn[:, b_].rearrange("p a d -> p (a d)"), ident,
                        )
                        nc.vector.tensor_copy(
                            out=y_t[:, 1, b_ * 128 : b_ * 128 + 128],
                            in_=tp[:, 0:128],
                        )
                        c_proj_tt(b_, last=True)

                    return [
                        lambda: tail_b(3),
                        lambda: tail_b(2),
                        lambda: tail_b(1),
                        lambda: tail_b(0),
                    ]
                return [
                    lambda: sweep(0),
                    lambda: sweep(1),
                    lambda: sweep(2),
                    lambda: (sweep(3), normalize()),
                    finalize,
                ]

            pending = []

            def block_pairs(j):
                # pair each diagonal block (as sub0, so its valid range
                # [c0:512] abuts sub1) with a non-diagonal block: the exp
                # range [c0:1024] stays contiguous -> one ACT op per tile
                nblk = 4 * j + 4
                diag = list(range(4 * j, nblk))
                nond = list(range(0, 4 * j))
                if not nond:
                    return [(0, 1), (2, 3)]
                pairs = list(zip(diag, nond[: len(diag)]))
                rest = nond[len(diag) :]
                return pairs + [(rest[k], rest[k + 1]) for k in range(0, len(rest), 2)]

            def emit_score_pair(j, h, i0, i1, p_map, chained):
                sp = ps_s.tile([128, 1024], F32, tag="s", name="ps_s")
                p_sb = pp.tile([128, 1024], BF16, tag="p", name="p_sb")
                for sub, i in ((0, i0), (1, i1)):
                    c0 = 128 * max(0, i - 4 * j)
                    nc.tensor.matmul(
                        sp[:, sub * 512 + c0 : sub * 512 + 512],
                        lhsT=kt[h][:, i * 128 : (i + 1) * 128],
                        rhs=qt[h][:, j * 512 + c0 : (j + 1) * 512],
                        start=True,
                        stop=True,
                    )
                    p_map[i] = (p_sb, sub)
                c00 = 128 * max(0, i0 - 4 * j)
                c01 = 128 * max(0, i1 - 4 * j)
                if c01 == 0:
                    e = nc.scalar.activation(
                        p_sb[:, c00:1024], sp[:, c00:1024], AF.Exp, bias=bias_m1
                    )
                else:  # j=0: both blocks diagonal, 2 ops
                    nc.scalar.activation(
                        p_sb[:, c00:512], sp[:, c00:512], AF.Exp, bias=bias_m1
                    )
                    e = nc.scalar.activation(
                        p_sb[:, 512 + c01 : 1024],
                        sp[:, 512 + c01 : 1024],
                        AF.Exp,
                        bias=bias_m1,
                    )
                if chained:
                    # keep this exp in the Phase A ACT chain so the
                    # scheduler can't interleave it with an Ln
                    ordered_act(e)
                # mask diagonal strips (Pool)
                for sub, i in ((0, i0), (1, i1)):
                    if 4 * j <= i <= 4 * j + 3:
                        boff = sub * 512 + 128 * (i - 4 * j)
                        nc.gpsimd.tensor_mul(
                            p_sb[:, boff : boff + 128],
                            p_sb[:, boff : boff + 128],
                            cmask,
                        )

            def score_pass(j, h):
                p_map = {}
                for i0, i1 in block_pairs(j):
                    emit_score_pair(j, h, i0, i1, p_map, chained=False)
                    # consume one flush stage of the previous (j, h)
                    # behind each S pair: the PE stream alternates score
                    # matmuls with y-sweep chunks and ACT stays fed
                    if pending:
                        pending.pop(0)()
                # carry at most the finalize stage into the next pass (the
                # P tiles are released once sweep3 ran, keeping pp bounded)
                while len(pending) > 1:
                    pending.pop(0)()
                pending.extend(flush_stages((j, h, p_map)))

            phaseA_pair(1)
            v_chunk(range(4, 8))
            for j, hs in ((1, HPCR), (2, HPCR), (3, HPCR), (0, HPCR)):
                if j in (2, 3):
                    v_chunk(range(4 * j, 4 * j + 4))
                for h in hs:
                    score_pass(j, h)
            while pending:
                pending.pop(0)()

    nc.compile()
    return nc


def make_inputs(x, w_attn, w_proj, delta):
    """Host-side prep: per-core input dicts (core = b*4 + g)."""
    x = np.asarray(x, dtype=np.float32)
    w_attn = np.asarray(w_attn, dtype=np.float32)
    w_proj = np.asarray(w_proj, dtype=np.float32)
    delta = np.asarray(delta, dtype=np.float32)
    bf = ml_dtypes.bfloat16

    inv_freq = 1.0 / (BASE ** (np.arange(D, dtype=np.float32) / D))
    t = np.arange(T, dtype=np.float32)
    freqs = t[:, None] * inv_freq[None, :]  # (T, D)
    scale = 1.0 / math.sqrt(D)
    trig = np.concatenate(
        [np.cos(freqs).T * scale, np.sin(freqs).T * scale], axis=0
    ).astype(bf)  # (128, T)

    d = np.clip(delta, -2.0 * math.pi, 0.0)

    qw = w_attn[:C].reshape(H, D, C)
    kw = w_attn[C : 2 * C].reshape(H, D, C)
    vw = w_attn[2 * C :].reshape(H, D, C)

    # causal mask for diagonal 128-blocks of P^T [tk, tq]: valid iff tq >= tk
    tk = np.arange(128)[:, None]
    cc = np.arange(128)[None, :]
    cmask = (cc >= tk).astype(bf)
    ident = np.eye(128, dtype=np.float32).astype(bf)

    in_maps = []
    for core in range(N_CORES):
        b, g = divmod(core, HPC)
        heads = range(HPC * g, HPC * g + HPC)

        xT = np.ascontiguousarray(x[b].T).reshape(NCT, 128, T).astype(bf)

        qk = np.stack(
            [np.concatenate([qw[h], kw[h]], axis=0) for h in heads], axis=0
        )  # (4, 128, C)
        wqk = np.ascontiguousarray(qk.transpose(2, 0, 1).reshape(C, 512)).reshape(
            NCT, 128, 512
        ).astype(bf)
        wv = np.ascontiguousarray(
            vw[HPC * g : HPC * g + HPC].reshape(256, C).T
        ).reshape(NCT, 128, 256).astype(bf)
        w2t = np.ascontiguousarray(
            w_proj[:, 256 * g : 256 * (g + 1)].T
        ).reshape(2, 128, 1024).astype(bf)

        ab = np.stack(
            [
                np.concatenate(
                    [
                        np.cos(freqs + d[h][None, :]).T,
                        np.sin(freqs + d[h][None, :]).T,
                    ],
                    axis=0,
                )
                for h in heads
            ],
            axis=0,
        ).astype(bf)  # (4, 128, T)

        in_maps.append(
            {
                "xT": xT,
                "wqk": wqk,
                "wv": wv,
                "w2": w2t,
                "trig": trig,
                "ab": ab,
                "cmask": cmask,
                "ident": ident,
            }
        )
    return in_maps


_NC_CACHE = []


def _get_nc():
    if not _NC_CACHE:
        _NC_CACHE.append(build_module())
    return _NC_CACHE[0]


def kernel(x, w_attn, w_proj, delta, _trace=False):
    in_maps = make_inputs(x, w_attn, w_proj, delta)
    nc = _get_nc()
    res = None
    outs = None
    last_err = None
    for attempt in range(3):
        try:
            res = bass_utils.run_bass_kernel_spmd(
                nc, in_maps, core_ids=list(range(N_CORES)), trace=_trace
            )
            outs = [
                np.asarray(r["out"]).reshape(T, C) for r in res.results
            ]
            break
        except Exception as e:
            last_err = e
            if "unrecoverable" not in str(e).lower() or attempt == 2:
                raise
            import time as _time

            _time.sleep(2.0)
    assert outs is not None, last_err
    if _trace:
        kernel.last_results = res
    full = np.zeros((B, T, C), dtype=np.float32)
    for core in range(N_CORES):
        full[core // HPC] += outs[core]
    return full


# revision 67
# speedup vs baseline: 1.2996x; 1.0332x over previous
"""Trainium2 Bass kernel for causal self-attention with PoPE (v2).

Reference computation (B=2, T=2048, C=1024, H=16, D=64):
  qkv = x @ w_attn.T ; split q,k,v ; heads
  mu_q = softplus(q); mu_k = softplus(k)
  q_real = mu_q * cos(t w); q_imag = mu_q * sin(t w)
  k_real = mu_k * cos(t w + d); k_imag = mu_k * sin(t w + d)
  att = softmax_causal((q_real k_real + q_imag k_imag)/sqrt(D))
  y = att @ v ; out = y @ w_proj.T

Sharding: 8 cores = 2 batches x 4 head-groups (4 heads each). Each core
computes its batch's QKV for its heads, attention, and a partial c_proj
(its heads' input-channel rows of w_proj). Host sums the 4 partials per
batch.

v2 design (132.9us vs the previous 172.7us version; ACT ~110us busy
is the bottleneck engine, PE ~100us):
  * all-bf16 tensors (the NEFF verifier rejects mixed f32r/bf16
    matmuls): halves DMA bytes, keeps every matmul at 1 cycle/row, and
    doubles DVE throughput on the elementwise path.
  * y matmul flipped to out[tq,d] (free dim 65 instead of 512): halves
    its PE cost AND lands the softmax denominator as a per-partition
    scalar (psum col 64), so normalization is a [128,4] reciprocal + 4
    tensor_scalar ops instead of reciprocal+PE-broadcast+mul. The sweep
    runs b-outer so only one psum accumulation group is open per bank
    (ZERO_REGION = a full 2KB bank).
  * y^T for c_proj via PE transpose (2 heads x 1 tq-128 block per op;
    gpsimd cannot read PSUM, so all psum->sbuf copies ride DVE/ACT).
  * c_proj fused per tq-128 block right after the last head's
    transpose: output DMAs spread across the whole attention phase; the
    very last pass runs per-b chains, largest b first, so the kernel
    tail is one short chain.
  * Phase A softplus = exp+ln batched per head-pair with explicit ACT
    ordering (the act-table pass otherwise reloads tables on every
    Exp<->Ln switch: 5 loads instead of 15); diagonal score blocks pair
    with non-diagonal ones so each psum tile needs a single contiguous
    exp.
  * every psum pool is opened once at the top (QK shares the S pool's
    tag) - no pool-boundary chaining stalls at phase transitions.
  * the (j,h) y-sweep is split into stages consumed one per S-pair of
    the following passes (up to 3 carried), so the in-order PE queue
    alternates score matmuls with sweep chunks and ACT never starves.
  * mu partition swaps via SBUF->SBUF DMA on the idle SP queue instead
    of DVE stream_shuffle; xT DMAs split per tbp-half across all three
    DMA-capable queues so the first QK psum completes ~6us in.
"""

import math
import os
import sys

import numpy as np

for _p in ("/opt/trn_rl_repo",):
    if _p not in sys.path and os.path.isdir(_p):
        sys.path.insert(0, _p)

import ml_dtypes

import concourse.tile as tile
from concourse import bacc
from concourse import mybir
from concourse import bass_utils

B, T, C = 2, 2048, 1024
H, D = 16, 64
BASE = 10000.0
N_CORES = 8
HPC = 4  # heads per core
NCT = 8  # c tiles (1024/128)
NTT = 16  # t tiles of 128

F32 = mybir.dt.float32
F32R = mybir.dt.float32r
BF16 = mybir.dt.bfloat16
AF = mybir.ActivationFunctionType


def build_module():
    nc = bacc.Bacc(
        "TRN2", target_bir_lowering=False, debug=False, num_devices=N_CORES
    )

    xT_d = nc.dram_tensor("xT", (NCT, 128, T), BF16, kind="ExternalInput").ap()
    wqk_d = nc.dram_tensor("wqk", (NCT, 128, 512), BF16, kind="ExternalInput").ap()
    wv_d = nc.dram_tensor("wv", (NCT, 128, 256), BF16, kind="ExternalInput").ap()
    w2_d = nc.dram_tensor("w2", (2, 128, 1024), BF16, kind="ExternalInput").ap()
    trig_d = nc.dram_tensor("trig", (128, T), BF16, kind="ExternalInput").ap()
    ab_d = nc.dram_tensor("ab", (HPC, 128, T), BF16, kind="ExternalInput").ap()
    cmask_d = nc.dram_tensor("cmask", (128, 128), BF16, kind="ExternalInput").ap()
    ident_d = nc.dram_tensor("ident", (128, 128), BF16, kind="ExternalInput").ap()
    out_d = nc.dram_tensor("out", (NTT, 128, 1024), F32, kind="ExternalOutput").ap()

    idm = list(range(32))

    with tile.TileContext(nc) as tc:
        with (
            tc.tile_pool(name="persist", bufs=1) as persist,
            tc.tile_pool(name="shufp", bufs=3) as shufp,
            tc.tile_pool(name="pp", bufs=12) as pp,
            tc.tile_pool(name="rcp", bufs=4) as rcp,
            tc.tile_pool(name="ynp", bufs=3) as ynp,
            tc.tile_pool(name="ostp", bufs=8) as ostp,
            tc.tile_pool(name="ps_s", bufs=2, space="PSUM") as ps_s,
            tc.tile_pool(name="ps_y", bufs=1, space="PSUM") as ps_y,
            tc.tile_pool(name="ps_t", bufs=1, space="PSUM") as ps_t,
            tc.tile_pool(name="ps_o", bufs=2, space="PSUM") as ps_o,
        ):
            # ---------------- persistent tiles ----------------
            v_aug = persist.tile([128, NTT, HPC, 65], BF16)
            nc.vector.memset(v_aug[:, :, :, 64:65], 1.0)
            trig = persist.tile([128, T], BF16)
            ab = persist.tile([128, HPC, T], BF16)
            cmask = persist.tile([128, 128], BF16)
            ident = persist.tile([128, 128], BF16)
            bias_m1 = persist.tile([128, 1], F32)
            nc.vector.memset(bias_m1, -1.0)
            mu = [persist.tile([128, T], BF16, name=f"mu{h}") for h in range(HPC)]
            qt = [persist.tile([128, T], BF16, name=f"qt{h}") for h in range(HPC)]
            kt = [persist.tile([128, T], BF16, name=f"kt{h}") for h in range(HPC)]
            y_t = persist.tile([128, 2, T], BF16)
            w2 = persist.tile([128, 2, 1024], BF16)
            wqk = persist.tile([128, NCT, 512], BF16)
            wv = persist.tile([128, NCT, 256], BF16)
            xT = persist.tile([128, NCT, T], BF16)

            # ------------- input DMAs (chunked; ACT only at idle start) ----
            xt_eng = {0: nc.sync, 3: nc.sync, 6: nc.sync,
                      1: nc.gpsimd, 4: nc.gpsimd, 7: nc.gpsimd,
                      2: nc.scalar, 5: nc.scalar}
            for c in range(NCT):
                weng = nc.gpsimd if c >= 6 else nc.sync
                weng.dma_start(wqk[:, c, :], wqk_d[c])
                # tbp-half granularity: the first QK psum only needs the
                # t<1024 halves, so its matmuls start ~2us earlier
                xt_eng[c].dma_start(xT[:, c, 0:1024], xT_d[c][:, 0:1024])
            for c in range(NCT):
                xt_eng[c].dma_start(xT[:, c, 1024:2048], xT_d[c][:, 1024:2048])
            nc.gpsimd.dma_start(wv, wv_d.rearrange("o p e -> p o e"))
            # trig/ab0-1 on the ACT queue: it is idle until the first QK
            # psum completes, so these transfers are free there; ab2-3
            # ride late on SP (needed only at prep h2/h3)
            nc.scalar.dma_start(trig, trig_d)
            for h in range(2):
                nc.scalar.dma_start(ab[:, h, :], ab_d[h])
            nc.gpsimd.dma_start(cmask, cmask_d)
            nc.sync.dma_start(ident, ident_d)
            nc.sync.dma_start(w2, w2_d.rearrange("o p e -> p o e"))
            for h in range(2, HPC):
                nc.sync.dma_start(ab[:, h, :], ab_d[h])

            # ---------------- Phase A: QKV projection ----------------
            # exp/ln batched per head-pair, with explicit ACT ordering so
            # the scheduler can't interleave Exp<->Ln (each switch costs a
            # 1283ns act-table reload): 5 loads total instead of 15.
            from concourse.tile_rust import add_dep_helper

            last_act = [None]

            def ordered_act(handle):
                if last_act[0] is not None:
                    add_dep_helper(
                        handle.ins, last_act[0].ins, sync=False,
                        reason="batch ACT ops to minimize table reloads",
                    )
                last_act[0] = handle

            def v_chunk(tts):
                # V tiles borrow the ps_o pool (temporally disjoint from
                # c_proj use: V at chunk start, c_proj at end)
                for tt in tts:
                    psv = ps_o.tile([128, 512], F32, tag="o", name="ps_o")
                    for c in range(NCT):
                        nc.tensor.matmul(
                            psv[:, 0:256],
                            lhsT=xT[:, c, tt * 128 : (tt + 1) * 128],
                            rhs=wv[:, c, :],
                            start=(c == 0),
                            stop=(c == NCT - 1),
                        )
                    nc.vector.tensor_copy(
                        out=v_aug[:, tt, :, 0:64],
                        in_=psv[:, 0:256].rearrange("p (h e) -> p h e", h=HPC),
                    )

            def phaseA_pair(hp, interleave=()):
                k = 0
                for h in (2 * hp, 2 * hp + 1):
                    for tbp in range(2):
                        ps = ps_s.tile([128, 1024], F32, tag="s", name="ps_s")
                        base = tbp * 1024
                        for c in range(NCT):
                            for half in range(2):
                                o0 = half * 512
                                nc.tensor.matmul(
                                    ps[:, o0 : o0 + 512],
                                    lhsT=wqk[:, c, h * 128 : (h + 1) * 128],
                                    rhs=xT[:, c, base + o0 : base + o0 + 512],
                                    start=(c == 0),
                                    stop=(c == NCT - 1),
                                )
                        ordered_act(
                            nc.scalar.activation(
                                mu[h][:, base : base + 1024], ps, AF.Exp
                            )
                        )
                        if k < len(interleave):
                            interleave[k]()
                            k += 1
                for h in (2 * hp, 2 * hp + 1):
                    ordered_act(
                        nc.scalar.activation(
                            mu[h], mu[h], AF.Ln, bias=1.0
                        )
                    )
                    # per-head preprocessing (overlaps later QK/V matmuls):
                    # qt rows 0:64 = mu_q cos(tw)/8, 64:128 = mu_q sin(tw)/8
                    # kt rows 0:64 = mu_k cos(tw+d), 64:128 = mu_k sin(tw+d)
                    # partition swap via SBUF->SBUF DMA on the idle SP queue
                    # (vs 2194ns of precious DVE per stream_shuffle), halved
                    # and interleaved so the first S matmuls start ~2us
                    # after the ln instead of ~5us
                    mks = shufp.tile([128, T], BF16, tag="mks", name="mks")
                    for th in range(2):
                        ts_ = slice(th * 1024, th * 1024 + 1024)
                        nc.sync.dma_start(mks[0:64, ts_], mu[h][64:128, ts_])
                        nc.sync.dma_start(mks[64:128, ts_], mu[h][0:64, ts_])
                        # mks-independent halves first (no DMA wait)
                        nc.gpsimd.tensor_mul(
                            qt[h][0:64, ts_], mu[h][0:64, ts_], trig[0:64, ts_]
                        )
                        nc.vector.tensor_mul(
                            kt[h][64:128, ts_], mu[h][64:128, ts_],
                            ab[64:128, h, ts_],
                        )
                        nc.vector.tensor_mul(
                            kt[h][0:64, ts_], mks[0:64, ts_], ab[0:64, h, ts_]
                        )
                        nc.gpsimd.tensor_mul(
                            qt[h][64:128, ts_], mks[64:128, ts_],
                            trig[64:128, ts_],
                        )

            # Phase A pairs with the j0 V chunks as PE filler while the
            # first pair's lns + preprocessing drain on ACT/DVE
            phaseA_pair(0)
            v_chunk(range(0, 4))

            # ------------- Phase B: attention + fused c_proj -------------
            HPCR = (0, 1, 2, 3)
            ypn_ref = [None]

            def c_proj_tt(tt, last=False):
                for eh in range(2):
                    po = ps_o.tile([128, 512], F32, tag="o", name="ps_o")
                    for ct in range(2):
                        nc.tensor.matmul(
                            po,
                            lhsT=y_t[:, ct, tt * 128 : (tt + 1) * 128],
                            rhs=w2[:, ct, eh * 512 : eh * 512 + 512],
                            start=(ct == 0),
                            stop=(ct == 1),
                        )
                    ost = ostp.tile([128, 512], F32, tag="ost", name="ost")
                    if last and eh == 1:
                        # final chunk: ACT is drained by now - use it for
                        # the copy (Copy is in every act table) + DMA so
                        # the tail isn't serialized on SP/Pool
                        nc.scalar.activation(ost, po, AF.Copy)
                        nc.scalar.dma_start(
                            out_d[tt][:, eh * 512 : eh * 512 + 512], ost
                        )
                    elif eh == 0:
                        nc.vector.tensor_copy(out=ost, in_=po)
                        nc.sync.dma_start(
                            out_d[tt][:, eh * 512 : eh * 512 + 512], ost
                        )
                    else:
                        nc.vector.tensor_copy(out=ost, in_=po)
                        nc.gpsimd.dma_start(
                            out_d[tt][:, eh * 512 : eh * 512 + 512], ost
                        )

            def flush_stages(pend):
                """Stages of the y-sweep + normalize (+transpose/c_proj)
                for a completed (j, h) score pass, to be interleaved one
                per S-pair of the next pass so the PE queue never sees a
                long non-score burst. b-outer keeps at most one pending
                psum accumulation group per bank."""
                jf, hf, p_map = pend
                yp = ps_y.tile([128, 4, 65], F32, tag="y", name="ps_y")

                def sweep(b_):
                    for i in range(4 * jf + b_ + 1):
                        tile_, sub = p_map[i]
                        nc.tensor.matmul(
                            yp[:, b_, :],
                            lhsT=tile_[
                                :, sub * 512 + b_ * 128 : sub * 512 + b_ * 128 + 128
                            ],
                            rhs=v_aug[:, i, hf, :],
                            start=(i == 0),
                            stop=(i == 4 * jf + b_),
                        )

                def normalize():
                    rc = rcp.tile([128, 4], F32, tag="rc", name="rc")
                    with nc.allow_low_precision(
                        reason="f32 reciprocal of softmax denominator"
                    ):
                        nc.vector.reciprocal(rc, yp[:, :, 64])
                    if hf % 2 == 0:
                        ypn_ref[0] = ynp.tile(
                            [128, 4, 2, 64], BF16, tag="yn", name="ypn"
                        )
                    ypn = ypn_ref[0]
                    for b_ in range(4):
                        nc.vector.tensor_scalar_mul(
                            ypn[:, b_, hf % 2, :], yp[:, b_, 0:64], rc[:, b_ : b_ + 1]
                        )

                def finalize():
                    if hf % 2 == 0:
                        return
                    # transpose 2 heads x [tq-128, 64] -> [128, tq-128]
                    hp = hf // 2
                    ypn = ypn_ref[0]
                    tp = ps_t.tile([128, 512], BF16, tag="t", name="ps_t")
                    for b_ in range(4):
                        nc.tensor.transpose(
                            tp[:, b_ * 128 : (b_ + 1) * 128],
                            ypn[:, b_].rearrange("p a d -> p (a d)"),
                            ident,
                        )
                    if hf == 3:
                        # per-b copy so c_proj(tt) streams out early
                        for b_ in range(4):
                            nc.vector.tensor_copy(
                                out=y_t[
                                    :, hp, jf * 512 + b_ * 128 : jf * 512 + b_ * 128 + 128
                                ],
                                in_=tp[:, b_ * 128 : (b_ + 1) * 128],
                            )
                            c_proj_tt(4 * jf + b_, last=(jf == 0))
                    else:
                        nc.vector.tensor_copy(
                            out=y_t[:, hp, jf * 512 : (jf + 1) * 512], in_=tp
                        )

                if jf == 0 and hf == 3:
                    # very last pass: per-b chains, largest b first, so
                    # the kernel tail is only b=0's short chain
                    def tail_b(b_):
                        sweep(b_)
                        rc = rcp.tile([128, 1], F32, tag="rc1", name="rc1")
                        with nc.allow_low_precision(
                            reason="f32 reciprocal of softmax denominator"
                        ):
                            nc.vector.reciprocal(rc, yp[:, b_, 64:65])
                        ypn = ypn_ref[0]
                        nc.vector.tensor_scalar_mul(
                            ypn[:, b_, 1, :], yp[:, b_, 0:64], rc
                        )
                        tp = ps_t.tile([128, 512], BF16, tag="t", name="ps_t")
                        nc.tensor.transpose(
                            tp[:, 0:128],
                            ypn[:, b_].rearrange("p a d -> p (a d)"), ident,
                        )
                        nc.vector.tensor_copy(
                            out=y_t[:, 1, b_ * 128 : b_ * 128 + 128],
                            in_=tp[:, 0:128],
                        )
                        c_proj_tt(b_, last=True)

                    return [
                        lambda: tail_b(3),
                        lambda: tail_b(2),
                        lambda: tail_b(1),
                        lambda: tail_b(0),
                    ]
                return [
                    lambda: sweep(0),
                    lambda: sweep(1),
                    lambda: sweep(2),
                    lambda: (sweep(3), normalize()),
                    finalize,
                ]

            pending = []

            def block_pairs(j):
                # pair each diagonal block (as sub0, so its valid range
                # [c0:512] abuts sub1) with a non-diagonal block: the exp
                # range [c0:1024] stays contiguous -> one ACT op per tile
                nblk = 4 * j + 4
                diag = list(range(4 * j, nblk))
                nond = list(range(0, 4 * j))
                if not nond:
                    return [(0, 1), (2, 3)]
                pairs = list(zip(diag, nond[: len(diag)]))
                rest = nond[len(diag) :]
                return pairs + [(rest[k], rest[k + 1]) for k in range(0, len(rest), 2)]

            def emit_score_pair(j, h, i0, i1, p_map, chained):
                sp = ps_s.tile([128, 1024], F32, tag="s", name="ps_s")
                p_sb = pp.tile([128, 1024], BF16, tag="p", name="p_sb")
                for sub, i in ((0, i0), (1, i1)):
                    c0 = 128 * max(0, i - 4 * j)
                    nc.tensor.matmul(
                        sp[:, sub * 512 + c0 : sub * 512 + 512],
                        lhsT=kt[h][:, i * 128 : (i + 1) * 128],
                        rhs=qt[h][:, j * 512 + c0 : (j + 1) * 512],
                        start=True,
                        stop=True,
                    )
                    p_map[i] = (p_sb, sub)
                c00 = 128 * max(0, i0 - 4 * j)
                c01 = 128 * max(0, i1 - 4 * j)
                if c01 == 0:
                    e = nc.scalar.activation(
                        p_sb[:, c00:1024], sp[:, c00:1024], AF.Exp, bias=bias_m1
                    )
                else:  # j=0: both blocks diagonal, 2 ops
                    nc.scalar.activation(
                        p_sb[:, c00:512], sp[:, c00:512], AF.Exp, bias=bias_m1
                    )
                    e = nc.scalar.activation(
                        p_sb[:, 512 + c01 : 1024],
                        sp[:, 512 + c01 : 1024],
                        AF.Exp,
                        bias=bias_m1,
                    )
                if chained:
                    # keep this exp in the Phase A ACT chain so the
                    # scheduler can't interleave it with an Ln
                    ordered_act(e)
                # mask diagonal strips (Pool)
                for sub, i in ((0, i0), (1, i1)):
                    if 4 * j <= i <= 4 * j + 3:
                        boff = sub * 512 + 128 * (i - 4 * j)
                        nc.gpsimd.tensor_mul(
                            p_sb[:, boff : boff + 128],
                            p_sb[:, boff : boff + 128],
                            cmask,
                        )

            def score_pass(j, h):
                p_map = {}
                for i0, i1 in block_pairs(j):
                    emit_score_pair(j, h, i0, i1, p_map, chained=False)
                    # consume one flush stage of the previous (j, h)
                    # behind each S pair: the PE stream alternates score
                    # matmuls with y-sweep chunks and ACT stays fed
                    if pending:
                        pending.pop(0)()
                # carry at most the finalize stage into the next pass (the
                # P tiles are released once sweep3 ran, keeping pp bounded)
                while len(pending) > 3:
                    pending.pop(0)()
                pending.extend(flush_stages((j, h, p_map)))

            phaseA_pair(1)
            v_chunk(range(4, 8))
            for j, hs in ((1, HPCR), (2, HPCR), (3, HPCR), (0, HPCR)):
                if j in (2, 3):
                    v_chunk(range(4 * j, 4 * j + 4))
                for h in hs:
                    score_pass(j, h)
            while pending:
                pending.pop(0)()

    nc.compile()
    return nc


def make_inputs(x, w_attn, w_proj, delta):
    """Host-side prep: per-core input dicts (core = b*4 + g)."""
    x = np.asarray(x, dtype=np.float32)
    w_attn = np.asarray(w_attn, dtype=np.float32)
    w_proj = np.asarray(w_proj, dtype=np.float32)
    delta = np.asarray(delta, dtype=np.float32)
    bf = ml_dtypes.bfloat16

    inv_freq = 1.0 / (BASE ** (np.arange(D, dtype=np.float32) / D))
    t = np.arange(T, dtype=np.float32)
    freqs = t[:, None] * inv_freq[None, :]  # (T, D)
    scale = 1.0 / math.sqrt(D)
    trig = np.concatenate(
        [np.cos(freqs).T * scale, np.sin(freqs).T * scale], axis=0
    ).astype(bf)  # (128, T)

    d = np.clip(delta, -2.0 * math.pi, 0.0)

    qw = w_attn[:C].reshape(H, D, C)
    kw = w_attn[C : 2 * C].reshape(H, D, C)
    vw = w_attn[2 * C :].reshape(H, D, C)

    # causal mask for diagonal 128-blocks of P^T [tk, tq]: valid iff tq >= tk
    tk = np.arange(128)[:, None]
    cc = np.arange(128)[None, :]
    cmask = (cc >= tk).astype(bf)
    ident = np.eye(128, dtype=np.float32).astype(bf)

    in_maps = []
    for core in range(N_CORES):
        b, g = divmod(core, HPC)
        heads = range(HPC * g, HPC * g + HPC)

        xT = np.ascontiguousarray(x[b].T).reshape(NCT, 128, T).astype(bf)

        qk = np.stack(
            [np.concatenate([qw[h], kw[h]], axis=0) for h in heads], axis=0
        )  # (4, 128, C)
        wqk = np.ascontiguousarray(qk.transpose(2, 0, 1).reshape(C, 512)).reshape(
            NCT, 128, 512
        ).astype(bf)
        wv = np.ascontiguousarray(
            vw[HPC * g : HPC * g + HPC].reshape(256, C).T
        ).reshape(NCT, 128, 256).astype(bf)
        w2t = np.ascontiguousarray(
            w_proj[:, 256 * g : 256 * (g + 1)].T
        ).reshape(2, 128, 1024).astype(bf)

        ab = np.stack(
            [
                np.concatenate(
                    [
                        np.cos(freqs + d[h][None, :]).T,
                        np.sin(freqs + d[h][None, :]).T,
                    ],
                    axis=0,
                )
                for h in heads
            ],
            axis=0,
        ).astype(bf)  # (4, 128, T)

        in_maps.append(
            {
                "xT": xT,
                "wqk": wqk,
                "wv": wv,
                "w2": w2t,
                "trig": trig,
                "ab": ab,
                "cmask": cmask,
                "ident": ident,
            }
        )
    return in_maps


_NC_CACHE = []


def _get_nc():
    if not _NC_CACHE:
        _NC_CACHE.append(build_module())
    return _NC_CACHE[0]


def kernel(x, w_attn, w_proj, delta, _trace=False):
    in_maps = make_inputs(x, w_attn, w_proj, delta)
    nc = _get_nc()
    res = None
    outs = None
    last_err = None
    for attempt in range(3):
        try:
            res = bass_utils.run_bass_kernel_spmd(
                nc, in_maps, core_ids=list(range(N_CORES)), trace=_trace
            )
            outs = [
                np.asarray(r["out"]).reshape(T, C) for r in res.results
            ]
            break
        except Exception as e:
            last_err = e
            if "unrecoverable" not in str(e).lower() or attempt == 2:
                raise
            import time as _time

            _time.sleep(2.0)
    assert outs is not None, last_err
    if _trace:
        kernel.last_results = res
    full = np.zeros((B, T, C), dtype=np.float32)
    for core in range(N_CORES):
        full[core // HPC] += outs[core]
    return full
